# revision 3
# baseline (speedup 1.0000x reference)
"""ActiveRotatingFilter gather kernel for 8 Trainium2 NeuronCores.

Semantics (matching the reference):
    idx = indices.reshape(72, 8) - 1
    inv = argsort(idx, axis=0)   (stable)
    out[o, r, i, e] = input[o, i, inv[e, r]]      out: [O*R, I*nOri, kH, kW]

Strategy: shard O=512 across 8 cores (64 planes each). Per core the input
shard (4.7 MB) lives in SBUF as [128 partitions = (o, i_hi), 9216 =
(i_lo, e)]. The graded exec window ends at the last DMA packet, and the
16-SDMA fabric tops out at ~433 GB/s shared between reads and writes, so
the whole kernel is scheduled to keep the DMA fabric saturated from the
first read chunk to the last write:

  - the input is read in 4 uneven free-dim chunks (il = 16/32/40/40),
    all queued up-front on the scalar HWDGE ring;
  - identity-rotation output chunks are interleaved right behind each
    read chunk on the sync ring so writes backfill fabric slack during
    the read phase;
  - the 7 permuted rotations are produced by VectorE (3 ring buffers,
    first rotation chunk-gated on the read stream), optionally with the
    last-but-one rotation offloaded to ScalarE/ACT (own SBUF port, but
    ~3.5x slower per element and it bank-conflicts with VectorE, so at
    most one rotation goes there).

Each permutation factors into a cyclic layer shift + a 9-element kernel
permutation, giving <=18 strided block copies per rotation.
"""

import numpy as np
from contextlib import ExitStack

O, I, NORI, KH, KW = 512, 256, 8, 3, 3
R = 8
E = NORI * KH * KW          # 72
NCORES = 8
O_SH = O // NCORES          # 64 output planes per core
P = 128                     # SBUF partitions, p = o*2 + i_hi
IL = I // 2                 # 128 i_lo values per partition
FD = IL * E                 # 9216 f32 per partition
IL_SPLITS = (16, 32, 40, 40)  # read chunk sizes along i_lo
N_ACT = 0                   # rotations offloaded to ScalarE/ACT (0 or 1)

_cache = {}


def _plan_rotation(col):
    """Decompose one permutation column into block-copy ops.

    Returns a list of ops:
      ("lgroup", s, j, qj): for all l: dst (l, j) <- src ((l - s) % 8, qj)
      ("run", a, b, ln):    dst [a, a+ln) <- src [b, b+ln)
    """
    col = col.astype(int)
    layers = col.reshape(NORI, KH * KW) // (KH * KW)
    q = col.reshape(NORI, KH * KW) % (KH * KW)
    structured = all(np.all(layers[l] == layers[l][0]) for l in range(NORI))
    if structured:
        l0 = layers[:, 0]
        s = int((-l0[0]) % NORI)
        structured = np.array_equal(l0, (np.arange(NORI) - s) % NORI) and all(
            np.array_equal(q[l], q[0]) for l in range(NORI)
        )
    if structured:
        return [("lgroup", s, j, int(q[0][j])) for j in range(KH * KW)]
    ops = []
    e = 0
    while e < E:
        b = int(col[e])
        ln = 1
        while e + ln < E and col[e + ln] == b + ln:
            ln += 1
        ops.append(("run", e, b, ln))
        e += ln
    return ops


def _emit_rotation_copies(copy, rot_plan, x_t, yt, sem, il_lo, il_hi, last):
    """Emit copies for one rotation, restricted to i_lo in [il_lo, il_hi).

    copy: the engine's copy method (vector.tensor_copy or scalar.copy).
    On the last instruction, then_inc(sem, 1) if last.
    """
    x4 = x_t[:].rearrange("p (il l j) -> p il l j", il=IL, l=NORI)
    y4 = yt[:].rearrange("p (il l j) -> p il l j", il=IL, l=NORI)
    x3 = x_t[:].rearrange("p (il e) -> p il e", il=IL)
    y3 = yt[:].rearrange("p (il e) -> p il e", il=IL)
    sl = slice(il_lo, il_hi)
    pairs = []
    for op in rot_plan:
        if op[0] == "lgroup":
            _, s, j, qj = op
            if s == 0:
                pairs.append((y4[:, sl, :, j], x4[:, sl, :, qj]))
            else:
                pairs.append((y4[:, sl, s:NORI, j], x4[:, sl, 0 : NORI - s, qj]))
                pairs.append((y4[:, sl, 0:s, j], x4[:, sl, NORI - s : NORI, qj]))
        else:
            _, a, b, ln = op
            pairs.append((y3[:, sl, a : a + ln], x3[:, sl, b : b + ln]))
    for i, (dst, src) in enumerate(pairs):
        instr = copy(dst, src)
        if last and i == len(pairs) - 1:
            instr.then_inc(sem, 1)


def _build(inv):
    import concourse.bass as bass
    import concourse.mybir as mybir

    f32 = mybir.dt.float32
    nc = bass.Bass("TRN2", target_bir_lowering=False, debug=False)
    x = nc.declare_dram_parameter("input", [P, FD], f32, isOutput=False)
    out = nc.declare_dram_parameter("out", [O_SH, R, 2, FD], f32, isOutput=True)

    ident = [r for r in range(R) if np.array_equal(inv[:, r], np.arange(E))]
    copies = [r for r in range(R) if r not in ident]
    rot_plans = {r: _plan_rotation(inv[:, r]) for r in copies}

    # chunk boundaries along il, as (il_lo, il_hi, flat_lo, flat_hi)
    cuts = [0]
    for s in IL_SPLITS:
        cuts.append(cuts[-1] + s)
    assert cuts[-1] == IL
    NCH = len(IL_SPLITS)
    chunks = [(cuts[c], cuts[c + 1], cuts[c] * E, cuts[c + 1] * E) for c in range(NCH)]

    # rotation -> producer: ACT gets the last-but-one rotation(s) (late
    # write deadline, no buffer reuse); DVE produces the rest.
    n_act = min(N_ACT, max(0, len(copies) - 1))
    act_rots = copies[-2:-1] if n_act else []
    dve_rots = [r for r in copies if r not in act_rots]

    # write order: identity chunks early (they backfill fabric slack
    # during the read phase), then rotations in production order.
    worder = []
    if ident:
        for c in range(NCH):
            worder.append(("id", ident[0], c))
    for r in copies:
        worder.append(("rot", r))
    for r in ident[1:]:
        for c in range(NCH):
            worder.append(("id", r, c))
    n_wr = len(worder)

    # write index (1-based) of each rotation's write, for y-buffer reuse
    wr_pos = {}
    for i, w in enumerate(worder):
        if w[0] == "rot":
            wr_pos[w[1]] = i + 1

    n_dve_buf = 3 if len(dve_rots) > 3 else max(1, len(dve_rots))
    n_yt = n_dve_buf + (1 if act_rots else 0)

    with ExitStack() as ctx:
        x_t = ctx.enter_context(nc.sbuf_tensor("x_t", [P, FD], f32))
        y_t = [
            ctx.enter_context(nc.sbuf_tensor(f"y_t{b}", [P, FD], f32))
            for b in range(n_yt)
        ]
        rd_sem = ctx.enter_context(nc.semaphore("rd_sem"))
        wr_sem = ctx.enter_context(nc.semaphore("wr_sem"))
        cpv_sem = ctx.enter_context(nc.semaphore("cpv_sem"))
        cpa_sem = ctx.enter_context(nc.semaphore("cpa_sem"))
        block = ctx.enter_context(nc.Block())

        buf_of = {}
        for k, r in enumerate(dve_rots):
            buf_of[r] = k % n_dve_buf
        for r in act_rots:
            buf_of[r] = n_dve_buf

        def emit_producer(eng, copy, rots, sem, nbuf):
            for k, r in enumerate(rots):
                yt = y_t[buf_of[r]]
                if k >= nbuf:
                    # y reuse: wait until the write that read this buffer
                    # has completed
                    prev = rots[k - nbuf]
                    eng.wait_ge(wr_sem, 16 * wr_pos[prev])
                if k == 0:
                    # chunk-gated so copies start while the input streams in
                    for c, (il_lo, il_hi, _, _) in enumerate(chunks):
                        eng.wait_ge(rd_sem, 16 * (c + 1))
                        _emit_rotation_copies(
                            copy, rot_plans[r], x_t, yt, sem,
                            il_lo, il_hi, last=(c == NCH - 1),
                        )
                else:
                    eng.wait_ge(rd_sem, 16 * NCH)
                    _emit_rotation_copies(
                        copy, rot_plans[r], x_t, yt, sem, 0, IL, last=True
                    )

        @block.scalar
        def _(scalar):
            # input load: all chunks queued up-front on the scalar ring
            for _, _, f_lo, f_hi in chunks:
                scalar.dma_start(
                    x_t[:, f_lo:f_hi], x[:, f_lo:f_hi]
                ).then_inc(rd_sem, 16)
            if act_rots:
                # warm the ACT function table (~1.5us) under the read phase
                scalar.copy(y_t[n_dve_buf][:, 0:1], x_t[:, 0:1])
                scalar.wait_ge(rd_sem, 16 * NCH)
                emit_producer(scalar, scalar.copy, act_rots, cpa_sem, 1)
            else:
                scalar.wait_ge(rd_sem, 16 * NCH)

        @block.sync
        def _(sync):
            for w in worder:
                if w[0] == "id":
                    _, r, c = w
                    il_lo, il_hi, f_lo, f_hi = chunks[c]
                    sync.wait_ge(rd_sem, 16 * (c + 1))
                    sync.dma_start(
                        out.ap()[:, r][:, :, f_lo:f_hi], x_t[:, f_lo:f_hi]
                    ).then_inc(wr_sem, 16)
                else:
                    r = w[1]
                    if r in dve_rots:
                        sync.wait_ge(cpv_sem, dve_rots.index(r) + 1)
                    else:
                        sync.wait_ge(cpa_sem, act_rots.index(r) + 1)
                    sync.dma_start(
                        out.ap()[:, r], y_t[buf_of[r]][:]
                    ).then_inc(wr_sem, 16)
            sync.wait_ge(wr_sem, 16 * n_wr)

        if dve_rots:
            @block.vector
            def _(vector):
                emit_producer(
                    vector, vector.tensor_copy, dve_rots, cpv_sem, n_dve_buf
                )

    return nc


def kernel(input, indices):
    from concourse.bass_utils import run_bass_kernel_spmd

    input = np.ascontiguousarray(np.asarray(input), dtype=np.float32)
    indices = np.asarray(indices)
    assert input.shape == (O, I, NORI, KH, KW), input.shape
    idx = indices.reshape(E, R).astype(np.int64) - 1
    inv = np.argsort(idx, axis=0, kind="stable")

    key = inv.tobytes()
    if key not in _cache:
        _cache[key] = _build(inv)
    nc = _cache[key]

    xs = input.reshape(O, I * E)
    in_maps = [
        {"input": np.ascontiguousarray(xs[c * O_SH : (c + 1) * O_SH]).reshape(P, FD)}
        for c in range(NCORES)
    ]
    res = run_bass_kernel_spmd(nc, in_maps, core_ids=list(range(NCORES)))
    parts = [res.results[c]["out"].reshape(O_SH, R, I, E) for c in range(NCORES)]
    full = np.concatenate(parts, axis=0)           # [O, R, I, E]
    return full.reshape(O * R, I * NORI, KH, KW)
